# revision 14
# baseline (speedup 1.0000x reference)
"""Trainium2 Bass kernel for nn_AbsoluteNeuralLayer.

Reference computation:
    classical = x @ classical_weights + classical_biases          # [B, DOUT]
    probs[j]  = |scan of circulant "rotations" applied to s0|[0]^2
    out       = tanh(classical + probs[None, :])

Key simplification: the scan state s0 is a constant vector, and every step
maps a constant vector to a constant vector scaled by cos(angle)
(s_new[i] = cos*s - sin*s + sin*s = cos*s elementwise).  Hence
    probs[j] = (prod_{t<48} cos(ang[j, t]))^2 / DIN
with ang[j, 3*d+g] = absolute_weights[d, j, g] for g < 3.

Sharding (8 cores): batch split 4 ways x dout split 2 ways.  Each core
computes out[1024 batch rows, 1024 dout cols] as tanh(x_s @ W_s + bias_s +
probs_s) with dout on PSUM partitions and batch on the moving free dim,
accumulating over K=2048 in 16 k-tiles.

v2+ (bf16, tuned schedule): the f32 version was HBM-bound (21 MB/core at
~330 GB/s/core while the PE floor is 54.6 us).  Storing x/W/out as bf16
halves traffic and makes the kernel PE-bound (rel err 1.163e-2 vs the
2e-2 gate, deterministic).  Measured-HW schedule notes:
  - dma_start (DIRECT2D) costs ~610 ns on the issuing sequencer and all
    rings share the 16 HW DMA queues (~330 GB/s steady, ~150-200 GB/s
    for the first few us, ~1.3 us issue->first-byte latency), so: all W
    on the sync ring in k-order (strict FIFO = earliest-deadline-first),
    16-nt chunks for fat 4 KB/partition descriptors, x0 on the gpsimd
    ring, pass-B x1 + out-tiles behind W on the sync ring, ang/bias on
    the gpsimd ring tail.
  - The HAM clock gate opens after ~3.5 us of sustained PE work (427 ns
    matmuls before, 216 ns after; <0.7 us gaps tolerated).  ~38 warmup
    matmuls on a memset tile keep the PE spinning through the DMA fill
    so the gate is already open when the first W/x chunks land (~12 us)
    and the real stream runs at full rate immediately.
  - pass A: m-chunk 0, k-outer over 8 PSUM banks; pass B: m-chunk 1 in
    [2,2,2,1,1] bank phases so pass A's ACT epilogues (0.69 us each)
    complete just ahead of pass B reusing the same banks.
  - Epilogue: ACT tanh (per-partition bias = probs+bias) drains PSUM to
    a bf16 tile; the final bank's out DMA issues on the scalar ring
    (same engine as the tanh) to shorten the kernel tail.
"""

import math

import numpy as np
import ml_dtypes

import concourse.bacc as bacc
import concourse.mybir as mybir
from concourse.tile import TileContext
from concourse.bass_utils import run_bass_kernel_spmd

B, DIN, DOUT, DEPTH = 4096, 2048, 2048, 16
NCORES = 8
BB, DB = 4, 2            # batch blocks x dout blocks (BB*DB == NCORES)
MB, NB = B // BB, DOUT // DB   # per-core batch rows (1024) / dout cols (1024)
KT = DIN // 128          # 16 contraction tiles
NT = NB // 128           # 8 dout tiles
MCH = 512                # batch chunk = one PSUM bank of fp32
MC = MB // MCH           # 2 chunks
NANG = 3 * DEPTH         # 48 angles per output column

# W stream chunks as (ring, n-tile count) in issue order over the
# (k,n)-major layout; x stream chunks in k-tile (MCH-col) units over the
# (u,k)-major layout.  rings: "s" = sync, "a" = scalar.
W_CHUNKS = [
    ("s", 8), ("s", 8), ("s", 8), ("s", 8),
    ("s", 16), ("s", 16), ("s", 16), ("s", 16), ("s", 16), ("s", 16),
]  # sum = KT*NT = 128; single ring => strict FIFO in EDF (k) order;
# k0..k3 as single-k chunks so the first matmul only waits for k0's bytes
# (warmup still spans the fill, so the HAM gate is open by then); 16-nt
# (4 KB/partition descriptors) for full HBM BW thereafter
X0_CHUNKS = [2, 1, 1, 4, 4, 4]                       # sum = KT = 16
X1_CHUNKS = [2, 2, 4, 8]                             # sum = KT = 16
B_SUBS = [2, 2, 2, 1, 1]                             # pass-B bank phases
WARMUP_MMS = 38

F32 = mybir.dt.float32
BF16 = mybir.dt.bfloat16
AF = mybir.ActivationFunctionType

_NC_CACHE = None


def _chunk_offsets(chunks):
    off, out = 0, []
    for c in chunks:
        out.append((off, c))
        off += c
    return out


def _build():
    nc = bacc.Bacc("TRN2", target_bir_lowering=False, debug=False, num_devices=NCORES)
    # host-packed SBUF layouts:
    #   wb [p, k*NB + n]          = W[128k+p, n]
    #   xb [p, (u*KT + k)*MCH+m]  = x[u*MCH + m, 128k+p]   (u = m-chunk)
    wb = nc.dram_tensor("wb", [128, KT * NB], BF16, kind="ExternalInput")
    xb = nc.dram_tensor("xb", [128, MC * KT * MCH], BF16, kind="ExternalInput")
    ang = nc.dram_tensor("ang", [128, NT * NANG], F32, kind="ExternalInput")
    bias = nc.dram_tensor("bias", [128, NT], F32, kind="ExternalInput")
    outT = nc.dram_tensor("outT", [NB, MB], BF16, kind="ExternalOutput")

    with TileContext(nc) as tc:
        with (
            tc.tile_pool(name="big", bufs=1) as big,
            tc.tile_pool(name="small", bufs=1) as small,
            tc.tile_pool(name="outp", bufs=8) as outp,
            tc.tile_pool(name="psum", bufs=1, space="PSUM") as psump,
        ):
            # ---- W stream (n-tile-unit chunks over sync + scalar rings) ----
            wg = {}  # (k, n) -> (tile, col offset)
            rings = {"s": nc.sync, "a": nc.scalar}
            t0 = 0
            for ci, (ring, tn) in enumerate(W_CHUNKS):
                wt = big.tile([128, tn * 128], BF16, tag=f"w{ci}", name=f"w{ci}")
                rings[ring].dma_start(out=wt, in_=wb[:, t0 * 128:(t0 + tn) * 128])
                for i in range(tn):
                    t = t0 + i
                    wg[(t // NT, t % NT)] = (wt, i * 128)
                t0 += tn

            # ---- x0 stream on the gpsimd ring (k-tile-unit chunks);
            # x1 (pass-B) issues LAST on the sync ring so its bytes queue
            # behind every pass-A-critical transfer (the 16 HW DMA queues
            # are shared across rings, ~306 GB/s aggregate) ----
            xs = {}  # (u, k) -> (tile, col offset)
            for ci, (c0, cn) in enumerate(_chunk_offsets(X0_CHUNKS)):
                xt = big.tile([128, cn * MCH], BF16, tag=f"x0_{ci}", name=f"x0_{ci}")
                nc.gpsimd.dma_start(out=xt, in_=xb[:, c0 * MCH:(c0 + cn) * MCH])
                for i in range(cn):
                    xs[(0, c0 + i)] = (xt, i * MCH)
            for ci, (c0, cn) in enumerate(_chunk_offsets(X1_CHUNKS)):
                xt = big.tile([128, cn * MCH], BF16, tag=f"x1_{ci}", name=f"x1_{ci}")
                nc.sync.dma_start(
                    out=xt, in_=xb[:, (KT + c0) * MCH:(KT + c0 + cn) * MCH]
                )
                for i in range(cn):
                    xs[(1, c0 + i)] = (xt, i * MCH)

            # ---- ang/bias on the gpsimd ring after x0 (tiny; feeds the
            # probs chain, which is only needed by the first epilogue) ----
            ang_sb = small.tile([128, NT * NANG], F32, tag="ang")
            nc.gpsimd.dma_start(out=ang_sb, in_=ang[:, :])
            bias_sb = small.tile([128, NT], F32, tag="bias")
            nc.gpsimd.dma_start(out=bias_sb, in_=bias[:, :])

            # ---- probs + bias compute (tiny, ACT/DVE) ----
            halfpi = small.tile([128, 1], F32, tag="halfpi")
            nc.vector.memset(halfpi, math.pi / 2)
            cos_sb = small.tile([128, NT * NANG], F32, tag="cos")
            nc.scalar.activation(cos_sb, ang_sb, AF.Sin, bias=halfpi)

            def v3(t):
                return t.rearrange("p (a b) -> p a b", a=NT)

            t24 = small.tile([128, NT * 24], F32, tag="t24")
            nc.vector.tensor_mul(v3(t24), v3(cos_sb)[:, :, 0:24], v3(cos_sb)[:, :, 24:48])
            t12 = small.tile([128, NT * 12], F32, tag="t12")
            nc.vector.tensor_mul(v3(t12), v3(t24)[:, :, 0:12], v3(t24)[:, :, 12:24])
            t6 = small.tile([128, NT * 6], F32, tag="t6")
            nc.vector.tensor_mul(v3(t6), v3(t12)[:, :, 0:6], v3(t12)[:, :, 6:12])
            t3 = small.tile([128, NT * 3], F32, tag="t3")
            nc.vector.tensor_mul(v3(t3), v3(t6)[:, :, 0:3], v3(t6)[:, :, 3:6])
            t1 = small.tile([128, NT], F32, tag="t1")
            nc.vector.tensor_mul(v3(t1), v3(t3)[:, :, 0:1], v3(t3)[:, :, 1:2])
            nc.vector.tensor_mul(v3(t1), v3(t1), v3(t3)[:, :, 2:3])
            sq = small.tile([128, NT], F32, tag="sq")
            nc.vector.tensor_mul(sq, t1, t1)
            nc.vector.tensor_scalar_mul(sq, sq, 1.0 / DIN)
            btot = small.tile([128, NT], F32, tag="btot")
            nc.vector.tensor_add(btot, sq, bias_sb)

            def mm_w(k, n):
                wt, off = wg[(k, n)]
                return wt[:, off:off + 128]

            def mm_x(u, k):
                xt, off = xs[(u, k)]
                return xt[:, off:off + MCH]

            def epilogue(n, ps_tile, u, ring=None):
                o = outp.tile([128, MCH], BF16, tag="o", name=f"o{n}_{u}")
                nc.scalar.activation(o, ps_tile, AF.Tanh, bias=btot[:, n:n + 1])
                (ring or nc.sync).dma_start(
                    out=outT[128 * n:128 * (n + 1), u * MCH:(u + 1) * MCH], in_=o
                )
                return o

            # ---- pass A: m-chunk 0, k-outer over 8 PSUM banks ----
            psA = [
                psump.tile([128, MCH], F32, tag=f"ps{n}", name=f"psA{n}")
                for n in range(NT)
            ]
            # PE warmup: dependency-free matmuls keep the PE busy (opening the
            # HAM clock gate) until the first W/x chunks land.
            warm = small.tile([128, 128], BF16, tag="warm")
            nc.vector.memset(warm, 0.0)
            for i in range(WARMUP_MMS):
                nc.tensor.matmul(psA[0][:, 0:128], warm, warm, start=True, stop=True)
            for k in range(KT):
                for n in range(NT):
                    nc.tensor.matmul(
                        psA[n], mm_w(k, n), mm_x(0, k),
                        start=(k == 0), stop=(k == KT - 1),
                    )

            # pass A epilogues (ACT) — free banks in n order for pass B
            for n in range(NT):
                epilogue(n, psA[n], 0)

            # ---- pass B: m-chunk 1, bank phases sized so each phase's
            # epilogues complete before the banks are reused ----
            n0 = 0
            for nsub in B_SUBS:
                psB = [
                    psump.tile(
                        [128, MCH], F32, tag=f"ps{n0 + t}", name=f"psB{n0 + t}"
                    )
                    for t in range(nsub)
                ]
                for k in range(KT):
                    for t in range(nsub):
                        nc.tensor.matmul(
                            psB[t], mm_w(k, n0 + t), mm_x(1, k),
                            start=(k == 0), stop=(k == KT - 1),
                        )
                for t in range(nsub):
                    last = n0 + t == NT - 1
                    o = epilogue(n0 + t, psB[t], 1, ring=nc.scalar if last else None)
                    if n0 + t == NT - 2:
                        # keep the DMA queues hot between bank 6's out and the
                        # final bank-7 out so the last transfer doesn't pay the
                        # ~0.9us queue-wake latency: dummy reads into bank 6's
                        # already-transferred tile (the WAR on its out-DMA
                        # times the fillers into exactly that idle window)
                        for fi in range(3):
                            nc.gpsimd.dma_start(
                                out=o, in_=wb[:, fi * MCH:(fi + 1) * MCH]
                            )
                n0 += nsub

    nc.compile()
    return nc


def _get_nc():
    global _NC_CACHE
    if _NC_CACHE is None:
        _NC_CACHE = _build()
    return _NC_CACHE


def _in_map_for_core(core, xbf, wbf, absolute_weights, classical_biases):
    i, j = core % BB, core // BB
    rows = slice(i * MB, (i + 1) * MB)
    cols = slice(j * NB, (j + 1) * NB)
    # wb[p, k*NB + n] = W[128k+p, n]
    wbm = np.ascontiguousarray(
        wbf[:, cols].reshape(KT, 128, NB).transpose(1, 0, 2).reshape(128, KT * NB)
    )
    # xb[p, (u*KT + k)*MCH + m] = x[rows][u*MCH+m, 128k+p]
    xsT = xbf[rows, :].T                                      # [DIN, MB] view
    xr = xsT.reshape(KT, 128, MC, MCH)                        # [k, p, u, m]
    xbm = np.ascontiguousarray(xr.transpose(1, 2, 0, 3).reshape(128, MC * KT * MCH))
    # ang[j_local, 3*d+g] = absolute_weights[d, j, g]
    angj = np.transpose(absolute_weights[:, cols, :3], (1, 0, 2)).reshape(NB, NANG)
    ang_sb = np.ascontiguousarray(
        angj.reshape(NT, 128, NANG).transpose(1, 0, 2).reshape(128, NT * NANG)
    )
    bias_sb = np.ascontiguousarray(classical_biases[cols].reshape(NT, 128).T)
    return {
        "wb": wbm,
        "xb": xbm,
        "ang": ang_sb.astype(np.float32, copy=False),
        "bias": bias_sb.astype(np.float32, copy=False),
    }


def kernel(x, absolute_weights, classical_weights, classical_biases, **_ignored):
    x = np.asarray(x, dtype=np.float32)
    absolute_weights = np.asarray(absolute_weights, dtype=np.float32)
    classical_weights = np.asarray(classical_weights, dtype=np.float32)
    classical_biases = np.asarray(classical_biases, dtype=np.float32)

    xbf = x.astype(ml_dtypes.bfloat16)
    wbf = classical_weights.astype(ml_dtypes.bfloat16)

    nc = _get_nc()
    in_maps = [
        _in_map_for_core(c, xbf, wbf, absolute_weights, classical_biases)
        for c in range(NCORES)
    ]
    res = run_bass_kernel_spmd(nc, in_maps, list(range(NCORES)))

    out = np.empty((B, DOUT), np.float32)
    for c in range(NCORES):
        i, j = c % BB, c // BB
        out[i * MB:(i + 1) * MB, j * NB:(j + 1) * NB] = (
            res.results[c]["outT"].T.astype(np.float32)
        )
    return out
